# revision 32
# baseline (speedup 1.0000x reference)
"""Mixtral sparse-MoE block (T=8192, H=1024, I=3584, E=8, top-2) on 8 TRN2 cores.

Strategy: expert-parallel with balanced block packing. The tiny gate (0.004%
of FLOPs) runs on host in fp64; tokens are dispatched (gathered) per expert
on host. Per-core capacity is NB blocks of 1024 tokens plus a Vt-token tail
slot (Vt minimized at runtime from the actual routing overflow, 0..128):

  - Weights stream from HBM once per block, so each block slot can hold a
    DIFFERENT expert's tokens at no extra DMA cost (host packs the per-core
    weight stream: w1p/w3p/w2t are [NS, ...] with one slot per block). Full
    1024-blocks and the largest per-expert remainders fill the 8*NB block
    slots; the leftover overflow (a few hundred tokens total) lands in the
    8 tail slots, so the skew of top-2 routing costs only ~Vt extra tokens
    per core instead of a whole extra 1024-block.
  - The tail slot has its own weight-stream slot and is interleaved into
    the last block's i-tile loop (phase 1 alongside, phase 2 at lag 1), so
    its full-weight stream spreads across the block's compute.

Each core computes the SwiGLU expert MLP over its compacted token batch:

    h = silu(x @ w1.T) * (x @ w3.T)        [*, I]
    y = combine_scale * (h @ w2.T)         [*, H]

and the host scatter-adds per-slot outputs back to [T, H].

Kernel matmuls default to bf16 (full PE rate at any moving size; halves
LDWEIGHTS and weight DMA vs fp32r). Per-block phases:
  phase 1 (per 128-wide I-tile): psum[i,t] = sum_h w1t[h,i]*xT[h,t]  (I on
    partitions, tokens moving, N=512 chunks) -> silu/mul -> h tile resident.
  phase 2 (per 128-token tile): psum[t,hh] = sum_i h[i,t]*w2t[i,hh]  (tokens
    on partitions) -> scale by combine weight -> DMA out. All 8 token tiles
    of the block accumulate at once (8 PSUM banks: 6 + the 2 phase-1 banks),
    so w2 streams exactly once per block.
Weights stream from HBM with host-prepacked layouts; every DMA is large and
contiguous-per-partition.
"""

import os
import sys
from contextlib import ExitStack

import numpy as np

for _p in ("/opt/trn_rl_repo", "/root/.axon_site/_ro/trn_rl_repo"):
    if os.path.isdir(_p) and _p not in sys.path:
        sys.path.insert(0, _p)
        break

T, H, I, E, TOPK = 8192, 1024, 3584, 8, 2
N_CORES = 8
P = 128
HC = H // P  # 8 contraction chunks of 128
IT = I // P  # 28 i-tiles

# matmul dtype: "bf16" (full rate at any N), "fp32r" (fp32 data, 11-bit
# mantissa, full rate only at N>=256), "fp32"
MM_DTYPE = os.environ.get("MOE_MM_DTYPE", "bf16")

_PROGRAM_CACHE: dict = {}


def _np_dt(dt_str):
    if dt_str == "bf16":
        import ml_dtypes

        return ml_dtypes.bfloat16
    return np.float32


def _round_fp32r(a):
    """Round fp32 -> fp32r (11-bit mantissa, RNE, low 12 bits zero) on host,
    matching walrus's fp32_to_fp32r. Assumes finite inputs."""
    u = np.ascontiguousarray(a, np.float32).view(np.uint32)
    lsb = (u >> 12) & 1
    u = (u + 0x7FF + lsb) & np.uint32(0xFFFFF000)
    return u.view(np.float32)


def _chunks(n, step):
    out, o = [], 0
    while o < n:
        out.append((o, min(step, n - o)))
        o += step
    return out


def _build_program(NB, Vt, dt_str):
    """Build + compile the SPMD Bass program: NB blocks of 8 tiles with
    per-block weight slots, plus a Vt-token tail slot with its own weight
    slot interleaved into the last block. Requires NB >= 1, 0 <= Vt <= 128."""
    import concourse.mybir as mybir
    import concourse.tile as tile
    from concourse import bacc

    key = (NB, Vt, dt_str)
    if key in _PROGRAM_CACHE:
        return _PROGRAM_CACHE[key]

    assert NB >= 1 and 0 <= Vt <= P

    DT = {
        "bf16": mybir.dt.bfloat16,
        "fp32r": mybir.dt.float32r,
        "fp32": mybir.dt.float32,
    }[dt_str]
    f32 = mybir.dt.float32
    NTm = NB * 8  # main tiles
    NTILES = NTm + (1 if Vt else 0)
    NHB = NB * 2  # x half-blocks of 512 tokens
    NS = NB + (1 if Vt else 0)  # weight-stream slots

    nc = bacc.Bacc("TRN2", target_bir_lowering=False, debug=False, num_devices=N_CORES)
    xt = nc.dram_tensor("xt", [NHB, P, HC * 512], DT, kind="ExternalInput").ap()
    if Vt:
        xv = nc.dram_tensor("xv", [P, HC * Vt], DT, kind="ExternalInput").ap()
    w1p = nc.dram_tensor("w1p", [NS, IT, P, HC * P], DT, kind="ExternalInput").ap()
    w3p = nc.dram_tensor("w3p", [NS, IT, P, HC * P], DT, kind="ExternalInput").ap()
    w2t = nc.dram_tensor("w2t", [NS, I, H], DT, kind="ExternalInput").ap()
    sc = nc.dram_tensor("sc", [NTILES, P, 1], f32, kind="ExternalInput").ap()
    y = nc.dram_tensor("y", [NTm * P + Vt, H], f32, kind="ExternalOutput").ap()

    Silu = mybir.ActivationFunctionType.Silu
    Copy = mybir.ActivationFunctionType.Copy

    with tile.TileContext(nc) as tc:
        with ExitStack() as ctx:
            xpool = ctx.enter_context(tc.tile_pool(name="xb", bufs=4))
            wpool = ctx.enter_context(tc.tile_pool(name="w13", bufs=3))
            w2pool = ctx.enter_context(tc.tile_pool(name="w2", bufs=4))
            hpool = ctx.enter_context(tc.tile_pool(name="h", bufs=IT))
            tmppool = ctx.enter_context(tc.tile_pool(name="tmp", bufs=3))
            ypool = ctx.enter_context(tc.tile_pool(name="y", bufs=4))
            spool = ctx.enter_context(tc.tile_pool(name="s", bufs=12))
            ps1 = ctx.enter_context(tc.tile_pool(name="ps1", bufs=1, space="PSUM"))
            ps2 = ctx.enter_context(tc.tile_pool(name="ps2", bufs=1, space="PSUM"))

            def psum_ph(which):  # phase-1 psum banks, shared with phase-2 py6/py7
                return ps1.tile([P, 512], f32, tag=which, name=which)

            def load_w13(slot, it, tagsuf="", split=False):
                w1s = wpool.tile(
                    [P, HC, P], DT, tag="w1s" + tagsuf, name="w1s" + tagsuf
                )
                w3s = wpool.tile(
                    [P, HC, P], DT, tag="w3s" + tagsuf, name="w3s" + tagsuf
                )
                src1 = w1p[slot, it].rearrange("p (c i) -> p c i", c=HC)
                src3 = w3p[slot, it].rearrange("p (c i) -> p c i", c=HC)
                if split:
                    # per-chunk loads so the first matmul group can start as
                    # soon as chunk 0 of w1/w3 has landed (kernel head latency)
                    for c in range(HC):
                        nc.sync.dma_start(w1s[:, c : c + 1, :], src1[:, c : c + 1, :])
                        nc.sync.dma_start(w3s[:, c : c + 1, :], src3[:, c : c + 1, :])
                else:
                    nc.sync.dma_start(w1s[:], src1)
                    nc.sync.dma_start(w3s[:], src3)
                return w1s, w3s

            def p1_chunks(w1s, w3s, xhbs, chunk_list, ht):
                # one I-tile of phase 1: psum = w1/w3 against the token chunks
                for ci, (tc0, tcs) in chunk_list:
                    xhb = xhbs[ci]
                    # w1 group first so silu(ph1) runs under the ph3 group and
                    # the ph3->mul->next-chunk chain never stalls the PE
                    ph1 = psum_ph("ph1")
                    ph3 = psum_ph("ph3")
                    for c in range(HC):
                        nc.tensor.matmul(
                            ph1[:, :tcs], w1s[:, c, :], xhb[:, c, :tcs],
                            start=(c == 0), stop=(c == HC - 1),
                        )
                    for c in range(HC):
                        nc.tensor.matmul(
                            ph3[:, :tcs], w3s[:, c, :], xhb[:, c, :tcs],
                            start=(c == 0), stop=(c == HC - 1),
                        )
                    sil = tmppool.tile([P, 512], f32, tag="sil", name="sil")
                    nc.scalar.activation(sil[:, :tcs], ph1[:, :tcs], Silu)
                    nc.vector.tensor_mul(
                        ht[:, tc0 : tc0 + tcs], sil[:, :tcs], ph3[:, :tcs]
                    )

            def load_xhbs(toff, tb, split_first=False):
                xhbs = []
                for j0, js in _chunks(tb, 512):
                    xhb = xpool.tile([P, HC, 512], DT, tag="xhb", name="xhb")
                    src = xt[(toff + j0) // 512].rearrange("p (c t) -> p c t", c=HC)
                    if split_first and j0 == 0:
                        # per-chunk loads so the first matmul group can start
                        # after one chunk instead of the full half-block
                        for c in range(HC):
                            nc.sync.dma_start(
                                xhb[:, c : c + 1, :js], src[:, c : c + 1, :js]
                            )
                    else:
                        nc.sync.dma_start(xhb[:, :, :js], src[:, :, :js])
                    xhbs.append(xhb)
                return xhbs

            def load_scales(g0, gn):
                stiles = []
                for tl in range(gn):
                    st = spool.tile([P, 1], f32, tag="s", name="st")
                    nc.sync.dma_start(st[:], sc[g0 + tl, :, :])
                    stiles.append(st)
                return stiles

            def store_y(tglob, hh, py, st, eng=0):
                # stores alternate between the ACT and DVE engines (~700ns
                # each reading PSUM) so a pass's store burst drains in half
                # the wall time and frees banks sooner
                yt = ypool.tile([P, 512], f32, tag="y", name="yt")
                if eng:
                    nc.vector.tensor_scalar_mul(yt[:], py[:], st[:])
                else:
                    nc.scalar.activation(yt[:], py[:], Copy, scale=st[:])
                nc.sync.dma_start(
                    y[tglob * P : (tglob + 1) * P, hh * 512 : (hh + 1) * 512], yt[:]
                )

            # PE warm-up under the initial DMA fill: ~5us of throwaway matmuls
            # trip the HAM activity window so the first real matmuls run at
            # 2.4GHz, and a 1-element Silu preloads the ACT table. Outputs are
            # never read.
            scr = tmppool.tile([P, 512], f32, tag="sil", name="scr")
            nc.any.memset(scr[:, :64], 0.0)
            nc.scalar.activation(scr[:, 64:65], scr[:, :1], Silu)
            pwarm = ps2.tile([P, 512], f32, tag="py0", name="pwarm")
            for _ in range(34):
                nc.tensor.matmul(
                    pwarm[:64, :64], scr[:, :64], scr[:, :64], start=True, stop=True
                )

            def phase2_block(g0, gn, hs, stiles, slot):
                # All gn tiles accumulate at once (8 PSUM banks: 6 + the 2
                # phase-1 banks), so w2 streams exactly once per pass. All
                # tiles finish together, so stores cluster at pass end; mm
                # order matches store order (descending) so the next pass's
                # bank waits are pipelined one store at a time.
                for hh in range(2):
                    pys = []
                    for tl in range(gn):
                        if tl < 6:
                            pys.append(
                                ps2.tile([P, 512], f32, tag=f"py{tl}", name=f"py{tl}")
                            )
                        else:  # borrow the phase-1 banks (idle during phase 2)
                            pys.append(psum_ph("ph1" if tl == 6 else "ph3"))
                    order = [6, 7, 4, 5, 2, 3, 0, 1]
                    tls = [t for t in order if t < gn]
                    for icp in range(0, IT, 2):  # paired w2 loads
                        npair = min(2, IT - icp)
                        w2s = w2pool.tile([P, 2, 512], DT, tag="w2s", name="w2s")
                        nc.sync.dma_start(
                            w2s[:, :npair],
                            w2t[
                                slot,
                                icp * P : (icp + npair) * P,
                                hh * 512 : (hh + 1) * 512,
                            ].rearrange("(a p) n -> p a n", p=P),
                        )
                        for a in range(npair):
                            ic = icp + a
                            for tl in tls:
                                nc.tensor.matmul(
                                    pys[tl][:],
                                    hs[ic][:, tl * P : (tl + 1) * P],
                                    w2s[:, a, :],
                                    start=(ic == 0), stop=(ic == IT - 1),
                                )
                    # borrowed banks first (ph1 = tl 6 leads, on DVE) so the
                    # next block's phase 1 w1 group unblocks after one store;
                    # adjacent pairs go to different engines and run abreast
                    for j, tl in enumerate(tls):
                        store_y(g0 + tl, hh, pys[tl], stiles[tl], eng=(j + 1) % 2)

            first = True
            for b in range(NB):
                toff = b * 8 * P
                merged = (b == NB - 1) and Vt > 0
                if first:
                    # first weight tile before x so the PE can start on
                    # chunk 0 as soon as the first tiles have landed
                    w13_0 = load_w13(0, 0)
                    xhbs = load_xhbs(toff, 8 * P, split_first=True)
                else:
                    xhbs = load_xhbs(toff, 8 * P)
                chunk_list = list(enumerate(_chunks(8 * P, 512)))
                if not merged:
                    hs = []
                    for it in range(IT):
                        w1s, w3s = (
                            w13_0 if (first and it == 0) else load_w13(b, it)
                        )
                        ht = hpool.tile([P, 8 * P], DT, tag="h", name="ht")
                        hs.append(ht)
                        p1_chunks(w1s, w3s, xhbs, chunk_list, ht)
                    stiles = load_scales(b * 8, 8)
                    phase2_block(b * 8, 8, hs, stiles, b)
                else:
                    # last block + Vt-token tail with its own weight slot; the
                    # tail's phase 2 interleaves at lag 1 inside phase 1
                    xvt = xpool.tile([P, HC, Vt], DT, tag="xtl", name="xtl", bufs=1)
                    nc.sync.dma_start(
                        xvt[:], xv.rearrange("p (c t) -> p c t", c=HC)
                    )
                    tail_chunks = [(0, (0, Vt))]
                    stiles = load_scales(b * 8, 8)
                    stv = load_scales(NTm, 1)[0]
                    pys_t = [
                        ps2.tile([P, 512], f32, tag=f"py{hh}", name=f"py{hh}")
                        for hh in range(2)
                    ]
                    hs = []
                    hts = []
                    for it in range(IT + 1):
                        if it < IT:
                            w1s, w3s = (
                                w13_0 if (first and it == 0) else load_w13(b, it)
                            )
                            ht = hpool.tile([P, 8 * P], DT, tag="h", name="ht")
                            hs.append(ht)
                            p1_chunks(w1s, w3s, xhbs, chunk_list, ht)
                            w1v, w3v = load_w13(NB, it, tagsuf="v")
                            htv = hpool.tile(
                                [P, Vt], DT, tag="htl", bufs=3, name="htl"
                            )
                            hts.append(htv)
                            p1_chunks(w1v, w3v, [xvt], tail_chunks, htv)
                        if it >= 1:
                            ic = it - 1
                            w2f = w2pool.tile([P, H], DT, tag="w2s", name="w2f")
                            nc.sync.dma_start(
                                w2f[:], w2t[NB, ic * P : (ic + 1) * P, :]
                            )
                            for hh in range(2):
                                nc.tensor.matmul(
                                    pys_t[hh][:Vt, :],
                                    hts[ic][:, :Vt],
                                    w2f[:, hh * 512 : (hh + 1) * 512],
                                    start=(ic == 0), stop=(ic == IT - 1),
                                )
                    for hh in range(2):
                        yt = ypool.tile([P, 512], f32, tag="y", name="yt")
                        if hh:
                            nc.vector.tensor_scalar_mul(
                                yt[:Vt, :], pys_t[hh][:Vt, :], stv[:Vt]
                            )
                        else:
                            nc.scalar.activation(
                                yt[:Vt, :], pys_t[hh][:Vt, :], Copy, scale=stv[:Vt]
                            )
                        nc.sync.dma_start(
                            y[NTm * P : NTm * P + Vt, hh * 512 : (hh + 1) * 512],
                            yt[:Vt, :],
                        )
                    phase2_block(b * 8, 8, hs, stiles, b)
                first = False

    nc.compile()
    _PROGRAM_CACHE[key] = nc
    return nc


def _route(x, gate_w, gate_b):
    """Top-2 routing on host, fp64 (verified to match the jax fp32 reference)."""
    lg = x.astype(np.float64) @ gate_w.T.astype(np.float64) + gate_b.astype(np.float64)
    lg -= lg.max(axis=-1, keepdims=True)
    p = np.exp(lg)
    p /= p.sum(axis=-1, keepdims=True)
    i1 = np.argmax(p, axis=-1)
    v1 = p[np.arange(p.shape[0]), i1]
    p2 = p.copy()
    p2[np.arange(p.shape[0]), i1] = -1.0
    i2 = np.argmax(p2, axis=-1)
    v2 = p2[np.arange(p2.shape[0]), i2]
    return (
        np.stack([i1, i2], 1),
        np.stack([v1, v2], 1).astype(np.float32),
    )


def _plan_assignment(cnts):
    """Pack per-expert token counts onto 8 cores at a common capacity.

    Returns (NB, Vt, blocks, vslots):
      blocks: list of 8*NB entries, each None or (expert, tok_lo, ntok<=1024)
      vslots: list of 8 entries, each None or (expert, tok_lo, ntok<=Vt)
    Capacity per core = NB blocks of 1024 tokens + one Vt-token tail slot.
    Every slot holds tokens of a single expert (so weight streams and output
    scatter stay per-expert). Vt is minimized so the tail compute shrinks to
    just the routing overflow.
    """
    NB = max(1, -(-sum(cnts) // (8 * 1024)))
    while True:
        fb = [c // 1024 for c in cnts]
        spare = 8 * NB - sum(fb)
        if spare < 0:
            NB += 1
            continue
        rem = [c - 1024 * b for c, b in zip(cnts, fb)]
        order = sorted(range(E), key=lambda e: -rem[e])
        in_block = set()
        for e in order:
            if spare <= 0 or rem[e] == 0:
                break
            in_block.add(e)
            spare -= 1
        ovf = {e: rem[e] for e in range(E) if rem[e] and e not in in_block}
        Vt = 0
        if ovf:
            for v in range(4, P + 1, 4):
                if sum(-(-o // v) for o in ovf.values()) <= 8:
                    Vt = v
                    break
            if Vt == 0:
                NB += 1
                continue
        blocks = []
        for e in range(E):
            for j in range(fb[e]):
                blocks.append((e, 1024 * j, 1024))
            if rem[e] and e in in_block:
                blocks.append((e, 1024 * fb[e], rem[e]))
        vslots = []
        for e, o in sorted(ovf.items(), key=lambda kv: -kv[1]):
            lo = 1024 * fb[e]
            done = 0
            while done < o:
                n = min(Vt, o - done)
                vslots.append((e, lo + done, n))
                done += n
        assert len(blocks) <= 8 * NB and len(vslots) <= 8
        blocks += [None] * (8 * NB - len(blocks))
        vslots += [None] * (8 - len(vslots))
        return NB, Vt, blocks, vslots


def _run_spmd(nc, in_maps, profile=False):
    from concourse import bass_utils

    core_ids = list(range(N_CORES))
    # First execution after NEFF load has shown sporadic stale-memory reads;
    # warm up, then run until two consecutive executions agree bitwise.
    bass_utils.run_bass_kernel_spmd(nc, in_maps, core_ids=core_ids)
    prev = None
    res = None
    for _ in range(4):
        res = bass_utils.run_bass_kernel_spmd(nc, in_maps, core_ids=core_ids)
        cur = [r["y"] for r in res.results]
        if prev is not None and all(
            np.array_equal(a, b) for a, b in zip(prev, cur)
        ):
            break
        prev = cur
    exec_ns = None
    if profile:
        pres = bass_utils.run_bass_kernel_spmd(
            nc, in_maps, core_ids=core_ids, trace=True
        )
        exec_ns = pres.exec_time_ns
    return res, exec_ns


def run(inputs, profile=False, dt_str=None):
    dt_str = dt_str or MM_DTYPE
    x = np.ascontiguousarray(np.asarray(inputs["x"], np.float32))
    gate_w = np.asarray(inputs["gate_w"], np.float32)
    gate_b = np.asarray(inputs["gate_b"], np.float32)
    w1 = np.asarray(inputs["w1"], np.float32)
    w2 = np.asarray(inputs["w2"], np.float32)
    w3 = np.asarray(inputs["w3"], np.float32)

    idx, val = _route(x, gate_w, gate_b)

    toks = []  # per-expert token index lists
    svals = []
    for e in range(E):
        m = idx == e  # [T, 2]
        sel = m.any(axis=1)
        te = np.nonzero(sel)[0]
        se = np.where(m[te, 0], val[te, 0], val[te, 1])
        toks.append(te)
        svals.append(se.astype(np.float32))
    NB, Vt, blocks, vslots = _plan_assignment([len(t) for t in toks])
    NTm = NB * 8
    NTILES = NTm + (1 if Vt else 0)
    NS = NB + (1 if Vt else 0)
    NHB = NB * 2

    npdt = _np_dt(dt_str)
    xT = np.ascontiguousarray(x.T)  # [H, T] fp32

    nc = _build_program(NB, Vt, dt_str)

    def conv(a):
        a = np.ascontiguousarray(a)
        if dt_str == "fp32r":
            return _round_fp32r(a)
        return a.astype(npdt)

    wpacks = []
    for e in range(E):
        # w1p/w3p slots: [IT, P, HC*P] with [it, p, c*P+ii] = w1[e].T[c*P+p, it*P+ii]
        w1pk = (
            w1[e].T.reshape(HC, P, IT, P).transpose(2, 1, 0, 3).reshape(IT, P, HC * P)
        )
        w3pk = (
            w3[e].T.reshape(HC, P, IT, P).transpose(2, 1, 0, 3).reshape(IT, P, HC * P)
        )
        wpacks.append((conv(w1pk), conv(w3pk), conv(w2[e].T)))
    zpack = (
        np.zeros((IT, P, HC * P), npdt),
        np.zeros((IT, P, HC * P), npdt),
        np.zeros((I, H), npdt),
    )

    def slot_tokens(slot):
        if slot is None:
            return None, np.zeros(0, np.int64), np.zeros(0, np.float32)
        e, lo, n = slot
        return e, toks[e][lo : lo + n], svals[e][lo : lo + n]

    in_maps = []
    core_slots = []  # per core: list of (row0, te) for output scatter
    for k in range(N_CORES):
        w1l, w3l, w2l, scat = [], [], [], []
        xg = np.zeros((H, NTm * P), np.float32)
        scp = np.zeros(NTILES * P, np.float32)
        for b in range(NB):
            slot = blocks[k * NB + b]
            e, te, se = slot_tokens(slot)
            p1, p3, p2w = wpacks[e] if e is not None else zpack
            w1l.append(p1)
            w3l.append(p3)
            w2l.append(p2w)
            r0 = b * 8 * P
            if len(te):
                xg[:, r0 : r0 + len(te)] = xT[:, te]
                scp[r0 : r0 + len(te)] = se
                scat.append((r0, te))
        # half-block packing: [NHB, P, HC*512], [j,p,c*512+t] = xg[c*P+p, j*512+t]
        xtp = (
            xg.reshape(HC, P, NHB, 512)
            .transpose(2, 1, 0, 3)
            .reshape(NHB, P, HC * 512)
        )
        im = dict(sc=scp.reshape(NTILES, P, 1))
        im["xt"] = conv(xtp)
        if Vt:
            e, te, se = slot_tokens(vslots[k])
            p1, p3, p2w = wpacks[e] if e is not None else zpack
            w1l.append(p1)
            w3l.append(p3)
            w2l.append(p2w)
            xgv = np.zeros((H, Vt), np.float32)
            if len(te):
                xgv[:, : len(te)] = xT[:, te]
                scp[NTm * P : NTm * P + len(te)] = se
                scat.append((NTm * P, te))
            im["xv"] = conv(
                xgv.reshape(HC, P, Vt).transpose(1, 0, 2).reshape(P, HC * Vt)
            )
        im["w1p"] = np.stack(w1l)
        im["w3p"] = np.stack(w3l)
        im["w2t"] = np.stack(w2l)
        in_maps.append(im)
        core_slots.append(scat)

    res, exec_ns = _run_spmd(nc, in_maps, profile=profile)

    out = np.zeros((T, H), np.float32)
    for k in range(N_CORES):
        yk = res.results[k]["y"]
        for r0, te in core_slots[k]:
            # one expert per slot -> token indices unique within a slot
            out[te] += yk[r0 : r0 + len(te)]
    return out, exec_ns


def kernel(**inputs):
    out, _ = run(inputs, profile=False)
    return out
